# revision 29
# baseline (speedup 1.0000x reference)
"""Trainium2 Bass kernel for nn_DataEmbedding_Stats.

Computation: rolling-window stats (window=24, replicate-padded) over
x (B,S,7) -> 35 features -> circular conv1d(k=3) -> (B,S,512).

Wave-pipelined design (8 NeuronCores, data parallel over batch, 4/core):
 - Host pre-transposes x to xp [S, 28] (col = 4c + b) so x loads are two
   112B-run DMAs; staged [128, 896] + PE transposes into X [128, 1047]
   f32 (partition = 32j + 4c + b, col = seq-in-chunk + 23 halo).
 - Positions advance in 3 ascending column-waves (512/384/128), computed
   simultaneously for all 4 chunks.  Per wave: log-doubling rolling
   sum/sumsq/max/min on DVE; finals + std land as bf16 in ST2A [128,
   4096] (x, mean=S24, max, min at partition 32t + 4c + b) / ST2B [28,
   4096] (std), col = position.
 - Relayout DMAs (per stat x chunk; full-width on the last wave) write
   straight into the k=1 rows (35..70) of F3 [106, 16384] bf16 (row =
   35k + 7t + c, col = 4096b + pos); k=0 / k=2 sections are built from
   the k=1 section by one shifted same-tile DMA per (wave, tap), plus
   tiny wrap patches for the circular conv.
 - conv as matmul per 128 positions: out[128,512] = F3[:, c:c+128].T @
   wt; matmuls stream per wave as their F3 columns complete.  PSUM
   pairs [128, 1024] -> one f32->bf16 copy (ACT/DVE alternating) into
   bf16 stage [128, 2048] -> 1KB-run DMAs to y (SP queue).
 - y stored bf16 (2e-2 rel-err budget), upcast to f32 on host.
"""

import numpy as np

try:
    import concourse.bass as bass  # noqa: F401
except ImportError:
    import sys

    for _p in ("/opt/trn_rl_repo", "/root/.axon_site/_ro/trn_rl_repo"):
        if _p not in sys.path:
            sys.path.insert(0, _p)

B, S, C, W, D = 32, 4096, 7, 24, 512
NCORES = 8
BSH = B // NCORES          # batches per core
NJ = 4                     # seq chunks (row groups of 32 partitions)
CH = S // NJ               # 1024
HALO = W - 1               # 23
XCOLS = CH + HALO          # 1047
NF = 5 * C                 # 35 features
K = 3 * NF + 1             # 106 contraction rows (ones row last)
ROWW = BSH * S             # 16384: F3 row width (col = 4096b + pos)
WB = (0, 384, 640, 896, 1024)  # wave boundaries (position within chunk)
NWAVE = len(WB) - 1
K1R = 35                   # F3 row offset of the k=1 (center-tap) section

_CACHE = {}


def _build():
    import concourse.bacc as bacc
    import concourse.tile as tile
    from concourse import mybir
    from concourse.ap import AP

    f32 = mybir.dt.float32
    bf16 = mybir.dt.float16
    Alu = mybir.AluOpType
    Act = mybir.ActivationFunctionType

    nc = bacc.Bacc(
        "TRN2",
        target_bir_lowering=False,
        debug=False,
        enable_asserts=False,
        num_devices=NCORES,
    )

    xp_d = nc.dram_tensor("xp", (S, 28), f32, kind="ExternalInput")
    xt_d = nc.dram_tensor("xt", (28, S), bf16, kind="ExternalInput")
    wt_d = nc.dram_tensor("wt", (K, D), bf16, kind="ExternalInput")
    ones_d = nc.dram_tensor("ones", (1, ROWW), bf16, kind="ExternalInput")
    id_d = nc.dram_tensor("ident", (128, 128), f32, kind="ExternalInput")
    y_d = nc.dram_tensor("y", (BSH, S, D), bf16, kind="ExternalOutput")

    with tile.TileContext(nc) as tc:
        with (
            tc.tile_pool(name="stats", bufs=1) as pst,
            tc.tile_pool(name="st2p", bufs=1) as pst2,
            tc.tile_pool(name="f3p", bufs=1) as pf3,
            tc.tile_pool(name="wtp", bufs=1) as pwt,
            tc.tile_pool(name="stage_in", bufs=1) as pstg,
            tc.tile_pool(name="psT", bufs=2, space="PSUM") as psT,
            tc.tile_pool(name="psum", bufs=3, space="PSUM") as pps,
            tc.tile_pool(name="outp", bufs=8) as pout,
        ):
            ident = pwt.tile([128, 128], f32, tag="ident")
            nc.sync.dma_start(ident[:], id_d.ap())
            wt = pwt.tile([K, D], bf16, tag="wt")
            F3 = pf3.tile([K, ROWW], bf16, tag="F3")
            ST2A = pst2.tile([128, S], bf16, tag="ST2A")
            ST2B = pst2.tile([28, S], bf16, tag="ST2B")
            f3h = F3[:].tensor
            xph = xp_d.ap().tensor
            xth = xt_d.ap().tensor
            yh = y_d.ap().tensor

            X = pst.tile([128, XCOLS], f32, tag="X")
            T1 = pst.tile([128, XCOLS], f32, tag="T1")
            T2 = pst.tile([128, XCOLS], f32, tag="T2")
            T3 = pst.tile([128, XCOLS], f32, tag="T3")
            T4 = pst.tile([128, XCOLS], f32, tag="T4")
            S24 = pst.tile([128, XCOLS], f32, tag="S24")
            T1B = pst.tile([128, XCOLS], bf16, tag="T1B")
            T2B = pst.tile([128, XCOLS], bf16, tag="T2B")
            T3B = pst.tile([128, XCOLS], bf16, tag="T3B")
            T4B = pst.tile([128, XCOLS], bf16, tag="T4B")
            S24B = pst.tile([128, XCOLS], bf16, tag="S24B")

            # ---- x: two DMAs (16 blocks each) + tail-halo block (SP)
            stg = pstg.tile([128, 896], f32, tag="stg")
            for h in range(2):
                nc.sync.dma_start(
                    AP(stg[:].tensor, 448 * h, [[896, 128], [28, 16], [1, 28]]),
                    AP(xph, 16 * 128 * 28 * h,
                       [[28, 128], [128 * 28, 16], [1, 28]]),
                )
            stgH = pstg.tile([23, 84], f32, tag="stgH")
            nc.sync.dma_start(
                AP(stgH[:].tensor, 0, [[84, 23], [28, 3], [1, 28]]),
                AP(xph, (CH - HALO) * 28, [[28, 23], [CH * 28, 3], [1, 28]]),
            )
            nc.sync.dma_start(wt[:], wt_d.ap())
            nc.sync.dma_start(F3[K - 1 : K, :], ones_d.ap())

            # chunk j+1 head halo <- chunk j tail (seq 1024j+1001..1023)
            for j in range(3):
                pstH = psT.tile([96, 128], f32, tag="pst")
                nc.tensor.transpose(pstH[0:28, 0:23],
                                    stgH[0:23, 28 * j : 28 * j + 28],
                                    ident[0:23, 0:23])
                nc.scalar.copy(
                    X[32 * (j + 1) : 32 * (j + 1) + 28, 0:HALO],
                    pstH[0:28, 0:23],
                )

            def transposes(w):
                for j in range(NJ):
                    for du in range(WB[w] // 128, WB[w + 1] // 128):
                        blk = 8 * j + du
                        pst = psT.tile([96, 128], f32, tag="pst")
                        nc.tensor.transpose(pst[0:28, :],
                                            stg[:, 28 * blk : 28 * blk + 28],
                                            ident[:])
                        c0 = HALO + 128 * du
                        nc.scalar.copy(X[32 * j : 32 * j + 28, c0 : c0 + 128],
                                       pst[0:28, :])

            transposes(0)
            # chunk-0 head halo: replicate x[b, 0, c] into cols 0..22
            nc.vector.tensor_scalar(
                X[0:28, 0:HALO], X[0:28, HALO : 2 * HALO], 0.0,
                X[0:28, HALO : HALO + 1], Alu.mult, Alu.add,
            )

            def tt(dst, d0, a, a0, bsrc, b0, n, op, eng=None):
                (eng or nc.vector).tensor_tensor(
                    dst[:, d0 : d0 + n], a[:, a0 : a0 + n],
                    bsrc[:, b0 : b0 + n], op
                )

            # relayout DMA: stat t, chunk j -> F3 k=1 rows (35 + 7t + c)
            _ri = [0]

            def emit_rearr(t, w, ow):
                # all 4 chunks of stat t in one DMA: the src/dst dims walk
                # as independent odometers (only total and last dim match)
                wl = WB[w + 1] - WB[w]
                if t == 0:
                    srcap = AP(xth, ow, [[S, 28], [CH, NJ], [1, wl]])
                elif t < 4:
                    srcap = AP(ST2A[:].tensor, 32 * t * S + ow,
                               [[S, 28], [CH, NJ], [1, wl]])
                else:
                    srcap = AP(ST2B[:].tensor, ow, [[S, 28], [CH, NJ], [1, wl]])
                dstap = AP(f3h, (K1R + 7 * t) * ROWW + ow,
                           [[ROWW, 7], [CH, 16], [1, wl]])
                eng = (nc.gpsimd, nc.sync)[_ri[0] % 2]
                _ri[0] += 1
                eng.dma_start(dstap, srcap)

            # k=0 / k=2 sections from the k=1 section (same-tile shift)
            def f3_build(w):
                ow, wl = WB[w], WB[w + 1] - WB[w]
                for k in (0, 2):
                    if k == 0:
                        dlo, slo = ow + 1, ow
                        L = wl if w < NWAVE - 1 else wl - 1
                    else:
                        if w == 0:
                            dlo, slo, L = 0, 1, wl - 1
                        else:
                            dlo, slo, L = ow - 1, ow, wl
                    nc.gpsimd.dma_start(
                        AP(f3h, 35 * k * ROWW + dlo,
                           [[ROWW, NF], [CH, 16], [1, L]]),
                        AP(f3h, K1R * ROWW + slo,
                           [[ROWW, NF], [CH, 16], [1, L]]),
                    )

            def patches_early():
                # k=2 dst col 1024j+1023 <- pos 1024(j+1) mod 4096
                for j in range(NJ):
                    nc.gpsimd.dma_start(
                        AP(f3h, 70 * ROWW + CH * j + CH - 1,
                           [[ROWW, NF], [S, BSH], [1, 1]]),
                        AP(f3h, K1R * ROWW + (CH * (j + 1)) % S,
                           [[ROWW, NF], [S, BSH], [1, 1]]),
                    )

            def patches_late():
                # k=0 dst col 1024j <- pos (1024j - 1) mod 4096
                for j in range(NJ):
                    nc.gpsimd.dma_start(
                        AP(f3h, CH * j, [[ROWW, NF], [S, BSH], [1, 1]]),
                        AP(f3h, K1R * ROWW + (CH * j - 1) % S,
                           [[ROWW, NF], [S, BSH], [1, 1]]),
                    )

            def stats_wave(w):
                ow = WB[w]
                WLW = WB[w + 1] - WB[w]
                D0 = HALO + ow
                E = D0 + WLW
                s1, s2, s3, s4, s5 = D0 - 22, D0 - 20, D0 - 16, D0 - 8, D0
                # stat 0: raw x straight from DRAM xT (bf16)
                emit_rearr(0, w, ow)
                # rolling sum (bf16, 2x DVE); mean folded into weights
                tt(T1B, s1, X, s1, X, s1 - 1, E - s1, Alu.add)
                tt(T2B, s2, T1B, s2, T1B, s2 - 2, E - s2, Alu.add)
                tt(T3B, s3, T2B, s3, T2B, s3 - 4, E - s3, Alu.add)
                tt(T1B, s4, T3B, s4, T3B, s4 - 8, E - s4, Alu.add)
                tt(S24B, s5, T1B, s5, T3B, s5 - 16, E - s5, Alu.add)
                # stat 1: mean (raw window sum)
                for j in range(NJ):
                    nc.scalar.copy(
                        ST2A[32:60, CH * j + ow : CH * j + ow + WLW],
                        S24B[32 * j : 32 * j + 28, s5:E],
                    )
                emit_rearr(1, w, ow)
                # squares (bf16)
                nc.scalar.square(T4B[:, s1 - 1 : E], X[:, s1 - 1 : E])
                tt(T1B, s1, T4B, s1, T4B, s1 - 1, E - s1, Alu.add)
                tt(T2B, s2, T1B, s2, T1B, s2 - 2, E - s2, Alu.add)
                tt(T3B, s3, T2B, s3, T2B, s3 - 4, E - s3, Alu.add)
                tt(T1B, s4, T3B, s4, T3B, s4 - 8, E - s4, Alu.add)
                tt(T2B, s5, T1B, s5, T3B, s5 - 16, E - s5, Alu.add)  # SQ24
                # var = max(SQ24 - S24^2/24, 0); std = sqrt(var/23) -> ST2B
                nc.scalar.activation(T4[:, s5:E], S24B[:, s5:E], Act.Square,
                                     0.0, float(W**-0.5))
                tt(T3, s5, T2B, s5, T4, s5, E - s5, Alu.subtract)
                nc.vector.tensor_scalar(T2[:, s5:E], T3[:, s5:E], 0.0, None,
                                        Alu.max)
                for j in range(NJ):
                    nc.scalar.activation(
                        ST2B[0:28, CH * j + ow : CH * j + ow + WLW],
                        T2[32 * j : 32 * j + 28, s5:E],
                        Act.Sqrt, 0.0, 1.0 / (W - 1),
                    )
                emit_rearr(4, w, ow)
                # max chain in bf16 (2x DVE); final -> ST2A rows 64..92
                tt(T1B, s1, X, s1, X, s1 - 1, E - s1, Alu.max)
                tt(T3B, s2, T1B, s2, T1B, s2 - 2, E - s2, Alu.max)
                tt(T1B, s3, T3B, s3, T3B, s3 - 4, E - s3, Alu.max)
                tt(T3B, s4, T1B, s4, T1B, s4 - 8, E - s4, Alu.max)
                for j in range(NJ):
                    nc.vector.tensor_tensor(
                        ST2A[64:92, CH * j + ow : CH * j + ow + WLW],
                        T3B[32 * j : 32 * j + 28, s5:E],
                        T1B[32 * j : 32 * j + 28, s5 - 16 : E - 16],
                        Alu.max,
                    )
                emit_rearr(2, w, ow)
                # min chain -> ST2A rows 96..124
                tt(T1B, s1, X, s1, X, s1 - 1, E - s1, Alu.min)
                tt(T3B, s2, T1B, s2, T1B, s2 - 2, E - s2, Alu.min)
                tt(T1B, s3, T3B, s3, T3B, s3 - 4, E - s3, Alu.min)
                tt(T3B, s4, T1B, s4, T1B, s4 - 8, E - s4, Alu.min)
                for j in range(NJ):
                    nc.vector.tensor_tensor(
                        ST2A[96:124, CH * j + ow : CH * j + ow + WLW],
                        T3B[32 * j : 32 * j + 28, s5:E],
                        T1B[32 * j : 32 * j + 28, s5 - 16 : E - 16],
                        Alu.min,
                    )
                emit_rearr(3, w, ow)

            def mm_group(rs):
                for b in range(BSH):
                    for r in rs:
                        stage = pout.tile([128, 4 * D], bf16, tag="stage")
                        for jp in range(2):
                            ps = pps.tile([128, 2 * D], f32, tag="ps")
                            for j2 in range(2):
                                j = 2 * jp + j2
                                c0 = S * b + CH * j + 128 * r
                                nc.tensor.matmul(
                                    ps[:, D * j2 : D * j2 + D],
                                    F3[:, c0 : c0 + 128],
                                    wt[:], start=True, stop=True)
                            ceng = (nc.scalar.copy
                                    if (2 * r + jp) % 2 == 0
                                    else nc.vector.tensor_copy)
                            ceng(stage[:, 2 * D * jp : 2 * D * jp + 2 * D],
                                 ps[:])
                        nc.sync.dma_start(
                            AP(yh, b * S * D + 128 * r * D,
                               [[D, 128], [CH * D, 4], [1, D]]),
                            AP(stage[:].tensor, 0,
                               [[4 * D, 128], [D, 4], [1, D]]),
                        )

            READY = {0: (1,), 1: (2, 3), 2: (4, 5), 3: (6, 7)}
            for w in range(NWAVE):
                stats_wave(w)
                if w + 1 < NWAVE:
                    transposes(w + 1)
                f3_build(w)
                if w == 0:
                    patches_early()
                if w == NWAVE - 1:
                    patches_late()
                mm_group(READY[w])
            mm_group((0,))

    nc.compile()
    return nc


def _prep_host(W_conv, b_conv):
    import ml_dtypes

    # F3 row = 35k + 7t + c  (tap-major); mean block /24 folded
    wkf = np.ascontiguousarray(W_conv.transpose(2, 1, 0)).copy()  # (3, 35, 512)
    wkf[:, C : 2 * C, :] *= 1.0 / W
    wt = np.empty((K, D), np.float32)
    wt[: K - 1] = wkf.reshape(3 * NF, D)
    wt[K - 1] = b_conv.astype(np.float32)
    return wt.astype(np.float16)


def _run(x, W_conv, b_conv, trace=False, **kw):
    from concourse import bass_utils

    if "nc" not in _CACHE:
        _CACHE["nc"] = _build()
    nc = _CACHE["nc"]

    wt = _prep_host(np.asarray(W_conv), np.asarray(b_conv))
    import ml_dtypes

    ones = np.ones((1, ROWW), np.float16)
    ident = np.eye(128, dtype=np.float32)
    x = np.ascontiguousarray(np.asarray(x, np.float32))
    in_maps = []
    for i in range(NCORES):
        xc = x[BSH * i : BSH * (i + 1)]          # (4, S, 7)
        xp = np.ascontiguousarray(xc.transpose(1, 2, 0)).reshape(S, 28)
        xt = np.ascontiguousarray(
            xc.transpose(2, 0, 1).reshape(28, S)).astype(np.float16)
        in_maps.append({"xp": xp, "xt": xt, "wt": wt, "ones": ones,
                        "ident": ident})
    res = bass_utils.run_bass_kernel_spmd(
        nc, in_maps, core_ids=list(range(NCORES)), trace=trace, **kw
    )
    out = np.concatenate(
        [np.asarray(r["y"], np.float32) for r in res.results], axis=0
    )
    return out, res


def kernel(x, x_mark=None, W_conv=None, b_conv=None, **_unused):
    out, _ = _run(x, W_conv, b_conv, trace=False)
    return out


# revision 32
# speedup vs baseline: 1.0071x; 1.0071x over previous
"""Trainium2 Bass kernel for nn_DataEmbedding_Stats.

Computation: rolling-window stats (window=24, replicate-padded) over
x (B,S,7) -> 35 features -> circular conv1d(k=3) -> (B,S,512).

Wave-pipelined design (8 NeuronCores, data parallel over batch, 4/core):
 - Host pre-transposes x to xp [S, 28] (col = 4c + b) so x loads are two
   112B-run DMAs; staged [128, 896] + PE transposes into X [128, 1047]
   f32 (partition = 32j + 4c + b, col = seq-in-chunk + 23 halo).
 - Positions advance in 3 ascending column-waves (512/384/128), computed
   simultaneously for all 4 chunks.  Per wave: log-doubling rolling
   sum/sumsq/max/min on DVE; finals + std land as bf16 in ST2A [128,
   4096] (x, mean=S24, max, min at partition 32t + 4c + b) / ST2B [28,
   4096] (std), col = position.
 - Relayout DMAs (per stat x chunk; full-width on the last wave) write
   straight into the k=1 rows (35..70) of F3 [106, 16384] bf16 (row =
   35k + 7t + c, col = 4096b + pos); k=0 / k=2 sections are built from
   the k=1 section by one shifted same-tile DMA per (wave, tap), plus
   tiny wrap patches for the circular conv.
 - conv as matmul per 128 positions: out[128,512] = F3[:, c:c+128].T @
   wt; matmuls stream per wave as their F3 columns complete.  PSUM
   pairs [128, 1024] -> one f32->bf16 copy (ACT/DVE alternating) into
   bf16 stage [128, 2048] -> 1KB-run DMAs to y (SP queue).
 - y stored bf16 (2e-2 rel-err budget), upcast to f32 on host.
"""

import numpy as np

try:
    import concourse.bass as bass  # noqa: F401
except ImportError:
    import sys

    for _p in ("/opt/trn_rl_repo", "/root/.axon_site/_ro/trn_rl_repo"):
        if _p not in sys.path:
            sys.path.insert(0, _p)

B, S, C, W, D = 32, 4096, 7, 24, 512
NCORES = 8
BSH = B // NCORES          # batches per core
NJ = 4                     # seq chunks (row groups of 32 partitions)
CH = S // NJ               # 1024
HALO = W - 1               # 23
XCOLS = CH + HALO          # 1047
NF = 5 * C                 # 35 features
K = 3 * NF + 1             # 106 contraction rows (ones row last)
ROWW = BSH * S             # 16384: F3 row width (col = 4096b + pos)
WB = (0, 384, 640, 896, 1024)  # wave boundaries (position within chunk)
NWAVE = len(WB) - 1
K1R = 35                   # F3 row offset of the k=1 (center-tap) section

_CACHE = {}


def _build():
    import concourse.bacc as bacc
    import concourse.tile as tile
    from concourse import mybir
    from concourse.ap import AP

    f32 = mybir.dt.float32
    bf16 = mybir.dt.float16
    Alu = mybir.AluOpType
    Act = mybir.ActivationFunctionType

    nc = bacc.Bacc(
        "TRN2",
        target_bir_lowering=False,
        debug=False,
        enable_asserts=False,
        num_devices=NCORES,
    )

    xp_d = nc.dram_tensor("xp", (S, 28), bf16, kind="ExternalInput")
    xt_d = nc.dram_tensor("xt", (28, S), bf16, kind="ExternalInput")
    wt_d = nc.dram_tensor("wt", (K, D), bf16, kind="ExternalInput")
    ones_d = nc.dram_tensor("ones", (1, ROWW), bf16, kind="ExternalInput")
    id_d = nc.dram_tensor("ident", (128, 128), bf16, kind="ExternalInput")
    y_d = nc.dram_tensor("y", (BSH, S, D), bf16, kind="ExternalOutput")

    with tile.TileContext(nc) as tc:
        with (
            tc.tile_pool(name="stats", bufs=1) as pst,
            tc.tile_pool(name="st2p", bufs=1) as pst2,
            tc.tile_pool(name="f3p", bufs=1) as pf3,
            tc.tile_pool(name="wtp", bufs=1) as pwt,
            tc.tile_pool(name="stage_in", bufs=1) as pstg,
            tc.tile_pool(name="psT", bufs=2, space="PSUM") as psT,
            tc.tile_pool(name="psum", bufs=3, space="PSUM") as pps,
            tc.tile_pool(name="outp", bufs=8) as pout,
        ):
            ident = pwt.tile([128, 128], bf16, tag="ident")
            nc.sync.dma_start(ident[:], id_d.ap())
            wt = pwt.tile([K, D], bf16, tag="wt")
            F3 = pf3.tile([K, ROWW], bf16, tag="F3")
            ST2A = pst2.tile([128, S], bf16, tag="ST2A")
            ST2B = pst2.tile([28, S], bf16, tag="ST2B")
            f3h = F3[:].tensor
            xph = xp_d.ap().tensor
            xth = xt_d.ap().tensor
            yh = y_d.ap().tensor

            X = pst.tile([128, XCOLS], bf16, tag="X")
            T1 = pst.tile([128, XCOLS], f32, tag="T1")
            T2 = pst.tile([128, XCOLS], f32, tag="T2")
            T3 = pst.tile([128, XCOLS], f32, tag="T3")
            T4 = pst.tile([128, XCOLS], f32, tag="T4")
            S24 = pst.tile([128, XCOLS], f32, tag="S24")
            T1B = pst.tile([128, XCOLS], bf16, tag="T1B")
            T2B = pst.tile([128, XCOLS], bf16, tag="T2B")
            T3B = pst.tile([128, XCOLS], bf16, tag="T3B")
            T4B = pst.tile([128, XCOLS], bf16, tag="T4B")
            S24B = pst.tile([128, XCOLS], bf16, tag="S24B")

            # ---- x: two DMAs (16 blocks each) + tail-halo block (SP)
            stg = pstg.tile([128, 896], bf16, tag="stg")
            for h in range(2):
                nc.sync.dma_start(
                    AP(stg[:].tensor, 448 * h, [[896, 128], [28, 16], [1, 28]]),
                    AP(xph, 16 * 128 * 28 * h,
                       [[28, 128], [128 * 28, 16], [1, 28]]),
                )
            stgH = pstg.tile([23, 84], bf16, tag="stgH")
            nc.sync.dma_start(
                AP(stgH[:].tensor, 0, [[84, 23], [28, 3], [1, 28]]),
                AP(xph, (CH - HALO) * 28, [[28, 23], [CH * 28, 3], [1, 28]]),
            )
            nc.sync.dma_start(wt[:], wt_d.ap())
            nc.sync.dma_start(F3[K - 1 : K, :], ones_d.ap())

            # chunk j+1 head halo <- chunk j tail (seq 1024j+1001..1023)
            for j in range(3):
                pstH = psT.tile([96, 128], bf16, tag="pst")
                nc.tensor.transpose(pstH[0:28, 0:23],
                                    stgH[0:23, 28 * j : 28 * j + 28],
                                    ident[0:23, 0:23])
                nc.scalar.copy(
                    X[32 * (j + 1) : 32 * (j + 1) + 28, 0:HALO],
                    pstH[0:28, 0:23],
                )

            def transposes(w):
                for j in range(NJ):
                    for du in range(WB[w] // 128, WB[w + 1] // 128):
                        blk = 8 * j + du
                        pst = psT.tile([96, 128], bf16, tag="pst")
                        nc.tensor.transpose(pst[0:28, :],
                                            stg[:, 28 * blk : 28 * blk + 28],
                                            ident[:])
                        c0 = HALO + 128 * du
                        nc.scalar.copy(X[32 * j : 32 * j + 28, c0 : c0 + 128],
                                       pst[0:28, :])

            transposes(0)
            # chunk-0 head halo: replicate x[b, 0, c] into cols 0..22
            nc.vector.tensor_copy(
                X[0:28, 0:HALO],
                X[0:28, HALO : HALO + 1].broadcast_to((28, HALO)),
            )

            def tt(dst, d0, a, a0, bsrc, b0, n, op, eng=None):
                (eng or nc.vector).tensor_tensor(
                    dst[:, d0 : d0 + n], a[:, a0 : a0 + n],
                    bsrc[:, b0 : b0 + n], op
                )

            # relayout DMA: stat t, chunk j -> F3 k=1 rows (35 + 7t + c)
            _ri = [0]

            def emit_rearr(t, w, ow):
                # all 4 chunks of stat t in one DMA: the src/dst dims walk
                # as independent odometers (only total and last dim match)
                wl = WB[w + 1] - WB[w]
                if t == 0:
                    srcap = AP(xth, ow, [[S, 28], [CH, NJ], [1, wl]])
                elif t < 4:
                    srcap = AP(ST2A[:].tensor, 32 * t * S + ow,
                               [[S, 28], [CH, NJ], [1, wl]])
                else:
                    srcap = AP(ST2B[:].tensor, ow, [[S, 28], [CH, NJ], [1, wl]])
                dstap = AP(f3h, (K1R + 7 * t) * ROWW + ow,
                           [[ROWW, 7], [CH, 16], [1, wl]])
                eng = (nc.gpsimd, nc.sync)[_ri[0] % 2]
                _ri[0] += 1
                eng.dma_start(dstap, srcap)

            # k=0 / k=2 sections from the k=1 section (same-tile shift)
            def f3_build(w):
                ow, wl = WB[w], WB[w + 1] - WB[w]
                for k in (0, 2):
                    if k == 0:
                        dlo, slo = ow + 1, ow
                        L = wl if w < NWAVE - 1 else wl - 1
                    else:
                        if w == 0:
                            dlo, slo, L = 0, 1, wl - 1
                        else:
                            dlo, slo, L = ow - 1, ow, wl
                    nc.gpsimd.dma_start(
                        AP(f3h, 35 * k * ROWW + dlo,
                           [[ROWW, NF], [CH, 16], [1, L]]),
                        AP(f3h, K1R * ROWW + slo,
                           [[ROWW, NF], [CH, 16], [1, L]]),
                    )

            def patches_early():
                # k=2 dst col 1024j+1023 <- pos 1024(j+1) mod 4096
                for j in range(NJ):
                    nc.gpsimd.dma_start(
                        AP(f3h, 70 * ROWW + CH * j + CH - 1,
                           [[ROWW, NF], [S, BSH], [1, 1]]),
                        AP(f3h, K1R * ROWW + (CH * (j + 1)) % S,
                           [[ROWW, NF], [S, BSH], [1, 1]]),
                    )

            def patches_late():
                # k=0 dst col 1024j <- pos (1024j - 1) mod 4096
                for j in range(NJ):
                    nc.gpsimd.dma_start(
                        AP(f3h, CH * j, [[ROWW, NF], [S, BSH], [1, 1]]),
                        AP(f3h, K1R * ROWW + (CH * j - 1) % S,
                           [[ROWW, NF], [S, BSH], [1, 1]]),
                    )

            def stats_wave(w):
                ow = WB[w]
                WLW = WB[w + 1] - WB[w]
                D0 = HALO + ow
                E = D0 + WLW
                s1, s2, s3, s4, s5 = D0 - 22, D0 - 20, D0 - 16, D0 - 8, D0
                # stat 0: raw x straight from DRAM xT (bf16)
                emit_rearr(0, w, ow)
                # rolling sum (bf16, 2x DVE); mean folded into weights
                tt(T1B, s1, X, s1, X, s1 - 1, E - s1, Alu.add)
                tt(T2B, s2, T1B, s2, T1B, s2 - 2, E - s2, Alu.add)
                tt(T3B, s3, T2B, s3, T2B, s3 - 4, E - s3, Alu.add)
                tt(T1B, s4, T3B, s4, T3B, s4 - 8, E - s4, Alu.add)
                tt(S24B, s5, T1B, s5, T3B, s5 - 16, E - s5, Alu.add)
                # stat 1: mean (raw window sum)
                for j in range(NJ):
                    nc.scalar.copy(
                        ST2A[32:60, CH * j + ow : CH * j + ow + WLW],
                        S24B[32 * j : 32 * j + 28, s5:E],
                    )
                emit_rearr(1, w, ow)
                # squares (bf16)
                nc.scalar.square(T4B[:, s1 - 1 : E], X[:, s1 - 1 : E])
                tt(T1B, s1, T4B, s1, T4B, s1 - 1, E - s1, Alu.add)
                tt(T2B, s2, T1B, s2, T1B, s2 - 2, E - s2, Alu.add)
                tt(T3B, s3, T2B, s3, T2B, s3 - 4, E - s3, Alu.add)
                tt(T1B, s4, T3B, s4, T3B, s4 - 8, E - s4, Alu.add)
                tt(T2B, s5, T1B, s5, T3B, s5 - 16, E - s5, Alu.add)  # SQ24
                # var = max(SQ24 - S24^2/24, 0); std = sqrt(var/23) -> ST2B
                nc.scalar.activation(T4[:, s5:E], S24B[:, s5:E], Act.Square,
                                     0.0, float(W**-0.5))
                tt(T3, s5, T2B, s5, T4, s5, E - s5, Alu.subtract)
                nc.vector.tensor_scalar(T2[:, s5:E], T3[:, s5:E], 0.0, None,
                                        Alu.max)
                for j in range(NJ):
                    nc.scalar.activation(
                        ST2B[0:28, CH * j + ow : CH * j + ow + WLW],
                        T2[32 * j : 32 * j + 28, s5:E],
                        Act.Sqrt, 0.0, 1.0 / (W - 1),
                    )
                emit_rearr(4, w, ow)
                # max chain in bf16 (2x DVE); final -> ST2A rows 64..92
                tt(T1B, s1, X, s1, X, s1 - 1, E - s1, Alu.max)
                tt(T3B, s2, T1B, s2, T1B, s2 - 2, E - s2, Alu.max)
                tt(T1B, s3, T3B, s3, T3B, s3 - 4, E - s3, Alu.max)
                tt(T3B, s4, T1B, s4, T1B, s4 - 8, E - s4, Alu.max)
                for j in range(NJ):
                    nc.vector.tensor_tensor(
                        ST2A[64:92, CH * j + ow : CH * j + ow + WLW],
                        T3B[32 * j : 32 * j + 28, s5:E],
                        T1B[32 * j : 32 * j + 28, s5 - 16 : E - 16],
                        Alu.max,
                    )
                emit_rearr(2, w, ow)
                # min chain -> ST2A rows 96..124
                tt(T1B, s1, X, s1, X, s1 - 1, E - s1, Alu.min)
                tt(T3B, s2, T1B, s2, T1B, s2 - 2, E - s2, Alu.min)
                tt(T1B, s3, T3B, s3, T3B, s3 - 4, E - s3, Alu.min)
                tt(T3B, s4, T1B, s4, T1B, s4 - 8, E - s4, Alu.min)
                for j in range(NJ):
                    nc.vector.tensor_tensor(
                        ST2A[96:124, CH * j + ow : CH * j + ow + WLW],
                        T3B[32 * j : 32 * j + 28, s5:E],
                        T1B[32 * j : 32 * j + 28, s5 - 16 : E - 16],
                        Alu.min,
                    )
                emit_rearr(3, w, ow)

            def mm_group(rs):
                for b in range(BSH):
                    for r in rs:
                        stage = pout.tile([128, 4 * D], bf16, tag="stage")
                        for jp in range(2):
                            ps = pps.tile([128, 2 * D], f32, tag="ps")
                            for j2 in range(2):
                                j = 2 * jp + j2
                                c0 = S * b + CH * j + 128 * r
                                nc.tensor.matmul(
                                    ps[:, D * j2 : D * j2 + D],
                                    F3[:, c0 : c0 + 128],
                                    wt[:], start=True, stop=True)
                            ceng = (nc.scalar.copy
                                    if (2 * r + jp) % 2 == 0
                                    else nc.vector.tensor_copy)
                            ceng(stage[:, 2 * D * jp : 2 * D * jp + 2 * D],
                                 ps[:])
                        nc.sync.dma_start(
                            AP(yh, b * S * D + 128 * r * D,
                               [[D, 128], [CH * D, 4], [1, D]]),
                            AP(stage[:].tensor, 0,
                               [[4 * D, 128], [D, 4], [1, D]]),
                        )

            READY = {0: (1,), 1: (2, 3), 2: (4, 5), 3: (6, 7)}
            for w in range(NWAVE):
                stats_wave(w)
                if w + 1 < NWAVE:
                    transposes(w + 1)
                f3_build(w)
                if w == 0:
                    patches_early()
                if w == NWAVE - 1:
                    patches_late()
                mm_group(READY[w])
            mm_group((0,))

    nc.compile()
    return nc


def _prep_host(W_conv, b_conv):
    import ml_dtypes

    # F3 row = 35k + 7t + c  (tap-major); mean block /24 folded
    wkf = np.ascontiguousarray(W_conv.transpose(2, 1, 0)).copy()  # (3, 35, 512)
    wkf[:, C : 2 * C, :] *= 1.0 / W
    wt = np.empty((K, D), np.float32)
    wt[: K - 1] = wkf.reshape(3 * NF, D)
    wt[K - 1] = b_conv.astype(np.float32)
    return wt.astype(np.float16)


def _run(x, W_conv, b_conv, trace=False, **kw):
    from concourse import bass_utils

    if "nc" not in _CACHE:
        _CACHE["nc"] = _build()
    nc = _CACHE["nc"]

    wt = _prep_host(np.asarray(W_conv), np.asarray(b_conv))
    import ml_dtypes

    ones = np.ones((1, ROWW), np.float16)
    ident = np.eye(128, dtype=np.float16)
    x = np.ascontiguousarray(np.asarray(x, np.float32))
    in_maps = []
    for i in range(NCORES):
        xc = x[BSH * i : BSH * (i + 1)]          # (4, S, 7)
        xp = np.ascontiguousarray(
            xc.transpose(1, 2, 0)).reshape(S, 28).astype(np.float16)
        xt = np.ascontiguousarray(
            xc.transpose(2, 0, 1).reshape(28, S)).astype(np.float16)
        in_maps.append({"xp": xp, "xt": xt, "wt": wt, "ones": ones,
                        "ident": ident})
    res = bass_utils.run_bass_kernel_spmd(
        nc, in_maps, core_ids=list(range(NCORES)), trace=trace, **kw
    )
    out = np.concatenate(
        [np.asarray(r["y"], np.float32) for r in res.results], axis=0
    )
    return out, res


def kernel(x, x_mark=None, W_conv=None, b_conv=None, **_unused):
    out, _ = _run(x, W_conv, b_conv, trace=False)
    return out
